# revision 1
# baseline (speedup 1.0000x reference)
"""Trainium2 Bass kernel for the GNN message-passing problem.

Math notes (why this is exact, not an approximation):
  score[i,b,j] = q[i,b]@wQKq + k[j,b]@wQKk + bQK.  Softmax over j is
  invariant to terms constant in j, so the attention weights are
  p[b,j] = softmax_j(nodes[j,b]@(WK.T@wQKk)) -- independent of the query
  node i.  Hence every node receives the same aggregated message and all
  Nn nodes are identical after one communicate() round; the final
  max-over-nodes is the common value.  Since p sums to 1 and V/A2N are
  affine, aggre@WV.T@WA.T collapses to hbar@(WA@WV).T with
  hbar[b] = sum_j p[b,j]*nodes[j,b].  Rounds 2-4 operate on identical
  nodes (uniform softmax == identity mix) and reduce to [B,H] math.

Sharding: data-parallel over batch B=32 across 8 cores (4 each).  The
only cross-batch coupling is the training-mode BatchNorm statistics, so
each core computes its pre-BN z (round 1) locally, a 1KB AllGather
replicates z to all cores, and the tiny [64,32] tail (4x BN+relu +
2 matmuls + heads) runs redundantly on every core.

Per-core on-chip layout: transposed activations [feature, row] with two
64-feature blocks packed on the 128 partitions (block A = batches 0,1 /
block B = batches 2,3 of the core's slice; 1536 columns per batch =
1535 input nodes + the out_enc node).
"""

import numpy as np

try:
    import concourse.bass as bass  # noqa: F401
except ImportError:  # pragma: no cover - container default path
    import sys

    sys.path.insert(0, "/opt/trn_rl_repo")
    import concourse.bass as bass  # noqa: F401

import concourse.bacc as bacc
import concourse.tile as tile
from concourse import mybir
from concourse import bass_utils

F32 = mybir.dt.float32
AF = mybir.ActivationFunctionType
ALU = mybir.AluOpType

NCORES = 8
B = 32
BL = B // NCORES  # 4 batch elements per core
N = 1535  # input nodes
NN = N + 1  # +1 out_enc node
H = 64
R2 = 2 * NN  # 3072 free columns (two batches per partition-block)
CH = 512
NCHK = R2 // CH  # 6 chunks
BN_EPS = 1e-5

# consts tensor column map ([128, CW] fp32)
C_W2 = 0  # [128,128] blockdiag(W2.T, W2.T)
C_WKE = 128  # [128,128] blockdiag(wk x ones, wk x ones)
C_W1 = 256  # [128,128] rows 0-8 cols 0-63 = W1.T ; rows 9-17 cols 64-127 = W1.T
C_WVA = 384  # [128,64] rows 0-63 = (WA@WV).T ; rows 64-127 = same
C_WENC = 448  # [128,64] rows 0-7 = Wenc.T
C_HEADS = 512  # [64,2] col0 = Wmu[0], col1 = Wsig[0]
C_B1 = 514  # [128,1] concat(b1,b1)
C_B2 = 515  # [128,1] concat(b2,b2)
C_GAMMA = 516  # [64,1]
C_BETA = 517  # [64,1]
C_BVA = 518  # [64,1] WA@bV + bA
C_BENC = 519  # [64,1]
C_BMU = 520  # row0 = bmu
C_BSIG = 521  # row0 = bsig
CW = 522


def build_nc(stage=99, noacc=False, mock_cc=False):
    """stage caps how much of the pipeline is emitted (debug bisect aid);
    the output tensor is written with whatever is available at that stage."""
    nc = bacc.Bacc(
        "TRN2",
        target_bir_lowering=False,
        debug=False,
        enable_asserts=True,
        num_devices=1 if mock_cc else NCORES,
    )
    xx = nc.dram_tensor("xx", [BL, N, 8], F32, kind="ExternalInput").ap()
    yy = nc.dram_tensor("yy", [BL, N], F32, kind="ExternalInput").ap()
    oxxT = nc.dram_tensor("oxxT", [8, BL], F32, kind="ExternalInput").ap()
    consts = nc.dram_tensor("consts", [128, CW], F32, kind="ExternalInput").ap()
    out = nc.dram_tensor("out", [1, 2 * B], F32, kind="ExternalOutput").ap()

    with tile.TileContext(nc) as tc:
        with (
            tc.tile_pool(name="big", bufs=1) as big,
            tc.tile_pool(name="small", bufs=1) as small,
            tc.tile_pool(name="prodp", bufs=2) as prodp,
            tc.tile_pool(name="psum_big", bufs=4, space="PSUM") as psum_big,
            tc.tile_pool(name="psum_small", bufs=2, space="PSUM") as psum_small,
            tc.tile_pool(name="dram", bufs=1, space="DRAM") as dram,
        ):
          def _body():
            # ---- input DMAs ----
            consts_sb = big.tile([128, CW], F32, tag="consts")
            nc.sync.dma_start(out=consts_sb[:], in_=consts)

            oxx_sb = small.tile([8, BL], F32, tag="oxx")
            nc.sync.dma_start(out=oxx_sb[:], in_=oxxT)

            xT = big.tile([18, R2], F32, tag="xT")
            # zero the out_enc columns (fc1/fc2 compute garbage there; it is
            # overwritten with the real encoding before use)
            nc.vector.memset(xT[:, N : N + 1], 0.0)
            nc.vector.memset(xT[:, NN + N : NN + N + 1], 0.0)
            xxT = xx.rearrange("b n k -> k b n")  # [8, BL, N]
            for blk in range(2):
                for bb in range(2):
                    b = 2 * blk + bb
                    nc.sync.dma_start(
                        out=xT[9 * blk : 9 * blk + 8, bb * NN : bb * NN + N],
                        in_=xxT[:, b, :],
                    )
                    nc.sync.dma_start(
                        out=xT[9 * blk + 8 : 9 * blk + 9, bb * NN : bb * NN + N],
                        in_=yy[b : b + 1, :],
                    )

            # ---- out_enc encoder: henc = relu(oxx @ Wenc.T + benc)  [64, BL] ----
            enc_ps = psum_small.tile([64, BL], F32, tag="sps")
            nc.tensor.matmul(
                enc_ps[:],
                consts_sb[0:8, C_WENC : C_WENC + 64],
                oxx_sb[:],
                start=True,
                stop=True,
            )
            henc = small.tile([64, BL], F32, tag="henc")
            nc.scalar.activation(
                out=henc[:],
                in_=enc_ps[:],
                func=AF.Relu,
                bias=consts_sb[0:64, C_BENC : C_BENC + 1],
                scale=1.0,
            )

            # ---- fc1: h1 = relu(x @ W1.T + b1), block-packed transposed ----
            h1T = big.tile([128, R2], F32, tag="h1T")
            for c in range(NCHK):
                sp = slice(c * CH, (c + 1) * CH)
                ps = psum_big.tile([128, CH], F32, tag="mm")
                nc.tensor.matmul(
                    ps[:],
                    consts_sb[0:18, C_W1 : C_W1 + 128],
                    xT[:, sp],
                    start=True,
                    stop=True,
                )
                if True:
                    nc.scalar.activation(
                        out=h1T[:, sp],
                        in_=ps[:],
                        func=AF.Relu,
                        bias=consts_sb[:, C_B1 : C_B1 + 1],
                        scale=1.0,
                    )
                else:
                    nc.vector.tensor_scalar(
                        h1T[:, sp],
                        ps[:],
                        consts_sb[:, C_B1 : C_B1 + 1],
                        0.0,
                        op0=ALU.add,
                        op1=ALU.max,
                    )

            if stage == 1:
                nc.sync.dma_start(out=out, in_=h1T[0:1, 0 : 2 * B])
                return
            # ---- fc2: h2 = relu(h1 @ W2.T + b2) ----
            h2T = big.tile([128, R2], F32, tag="h2T")
            for c in range(NCHK):
                sp = slice(c * CH, (c + 1) * CH)
                ps = psum_big.tile([128, CH], F32, tag="mm")
                nc.tensor.matmul(
                    ps[:],
                    consts_sb[:, C_W2 : C_W2 + 128],
                    h1T[:, sp],
                    start=True,
                    stop=True,
                )
                if True:
                    nc.scalar.activation(
                        out=h2T[:, sp],
                        in_=ps[:],
                        func=AF.Relu,
                        bias=consts_sb[:, C_B2 : C_B2 + 1],
                        scale=1.0,
                    )
                else:
                    nc.vector.tensor_scalar(
                        h2T[:, sp],
                        ps[:],
                        consts_sb[:, C_B2 : C_B2 + 1],
                        0.0,
                        op0=ALU.add,
                        op1=ALU.max,
                    )

            # place out_enc encodings at node column 1535 of each batch
            nc.vector.tensor_copy(out=h2T[0:64, N : N + 1], in_=henc[:, 0:1])
            nc.vector.tensor_copy(
                out=h2T[0:64, NN + N : NN + N + 1], in_=henc[:, 1:2]
            )
            # cross-partition placement (block B lives on partitions 64-127)
            nc.sync.dma_start(out=h2T[64:128, N : N + 1], in_=henc[:, 2:3])
            nc.sync.dma_start(
                out=h2T[64:128, NN + N : NN + N + 1], in_=henc[:, 3:4]
            )

            if stage == 2:
                nc.sync.dma_start(out=out, in_=h2T[0:1, 0 : 2 * B])
                return
            # ---- attention scores broadcast + exp ----
            # skb[m, n] = sum_k wk[k] * h2T[k, n] for k in block(m): every
            # partition of a block holds that block's sk row, so Exp runs at
            # full width and accum_out yields the softmax denominator.
            ebc = big.tile([128, R2], F32, tag="ebc")
            dacc = [
                small.tile([128, 1], F32, tag=f"dacc{c}", name=f"dacc{c}")
                for c in range(NCHK)
            ]
            for c in range(NCHK):
                sp = slice(c * CH, (c + 1) * CH)
                ps = psum_big.tile([128, CH], F32, tag="mm")
                nc.tensor.matmul(
                    ps[:],
                    consts_sb[:, C_WKE : C_WKE + 128],
                    h2T[:, sp],
                    start=True,
                    stop=True,
                )
                nc.scalar.activation(
                    out=ebc[:, sp],
                    in_=ps[:],
                    func=AF.Exp,
                    bias=0.0,
                    scale=1.0,
                )
                # ACT accum_out faults the device on this stack; reduce on DVE
                nc.vector.tensor_reduce(
                    out=dacc[c][:],
                    in_=ebc[:, sp],
                    axis=mybir.AxisListType.X,
                    op=ALU.add,
                )

            if stage == 3:
                nc.sync.dma_start(out=out, in_=ebc[0:1, 0 : 2 * B])
                return
            # softmax denominators per batch (replicated per partition)
            dA1 = small.tile([128, 1], F32, tag="dA1")
            dA = small.tile([128, 1], F32, tag="dA")
            nc.vector.tensor_add(dA1[:], dacc[0][:], dacc[1][:])
            nc.vector.tensor_add(dA[:], dA1[:], dacc[2][:])
            dB1 = small.tile([128, 1], F32, tag="dB1")
            dB = small.tile([128, 1], F32, tag="dB")
            nc.vector.tensor_add(dB1[:], dacc[3][:], dacc[4][:])
            nc.vector.tensor_add(dB[:], dB1[:], dacc[5][:])
            rdA = small.tile([128, 1], F32, tag="rdA")
            rdB = small.tile([128, 1], F32, tag="rdB")
            nc.vector.reciprocal(out=rdA[:], in_=dA[:])
            nc.vector.reciprocal(out=rdB[:], in_=dB[:])

            if stage == 31:
                nc.sync.dma_start(out=out[0:1, 0:1], in_=rdA[0:1, :])
                return
            # ---- weighted node sums: num = sum_j e[j] * h2[:, j] ----
            # tensor_tensor_reduce faults this stack; use mul + reduce
            numacc = [
                small.tile([128, 1], F32, tag=f"numacc{c}", name=f"numacc{c}")
                for c in range(NCHK)
            ]
            for c in range(NCHK):
                sp = slice(c * CH, (c + 1) * CH)
                prod = prodp.tile([128, CH], F32, tag="prod")
                nc.vector.tensor_mul(prod[:], h2T[:, sp], ebc[:, sp])
                nc.vector.tensor_reduce(
                    out=numacc[c][:],
                    in_=prod[:],
                    axis=mybir.AxisListType.X,
                    op=ALU.add,
                )
            nA1 = small.tile([128, 1], F32, tag="nA1")
            numA = small.tile([128, 1], F32, tag="numA")
            nc.vector.tensor_add(nA1[:], numacc[0][:], numacc[1][:])
            nc.vector.tensor_add(numA[:], nA1[:], numacc[2][:])
            nB1 = small.tile([128, 1], F32, tag="nB1")
            numB = small.tile([128, 1], F32, tag="numB")
            nc.vector.tensor_add(nB1[:], numacc[3][:], numacc[4][:])
            nc.vector.tensor_add(numB[:], nB1[:], numacc[5][:])

            # hbar = num / d
            hbA = small.tile([128, 1], F32, tag="hbA")
            hbB = small.tile([128, 1], F32, tag="hbB")
            nc.vector.tensor_scalar_mul(hbA[:], numA[:], rdA[:])
            nc.vector.tensor_scalar_mul(hbB[:], numB[:], rdB[:])

            if stage == 35:
                nc.sync.dma_start(out=out[0:1, 0:1], in_=hbA[0:1, :])
                return
            # ---- z = hbar @ (WA@WV).T + bva   -> [64, BL] ----
            z_ps = psum_small.tile([64, BL], F32, tag="sps")
            va0 = consts_sb[0:64, C_WVA : C_WVA + 64]
            va1 = consts_sb[64:128, C_WVA : C_WVA + 64]
            nc.tensor.matmul(z_ps[:, 0:1], va0, hbA[0:64, :], start=True, stop=True)
            nc.tensor.matmul(z_ps[:, 1:2], va0, hbB[0:64, :], start=True, stop=True)
            nc.tensor.matmul(z_ps[:, 2:3], va1, hbA[64:128, :], start=True, stop=True)
            nc.tensor.matmul(z_ps[:, 3:4], va1, hbB[64:128, :], start=True, stop=True)
            z_sb = small.tile([64, BL], F32, tag="z_sb")
            nc.scalar.activation(
                out=z_sb[:],
                in_=z_ps[:],
                func=AF.Identity,
                bias=consts_sb[0:64, C_BVA : C_BVA + 1],
                scale=1.0,
            )

            if stage == 4:
                nc.sync.dma_start(out=out[0:1, 0:BL], in_=z_sb[0:1, :])
                return
            # ---- AllGather pre-BN z across the 8 cores ----
            cc_in = dram.tile([64, BL], F32, tag="cc_in")
            cc_out = dram.tile([NCORES, 64, BL], F32, tag="cc_out")
            nc.sync.dma_start(out=cc_in[:], in_=z_sb[:])
            if mock_cc:
                # single-core timing build: stand in for the AllGather
                nc.sync.dma_start(out=cc_out[0], in_=cc_in[:])
            else:
                nc.gpsimd.collective_compute(
                    "AllGather",
                    ALU.bypass,
                    replica_groups=[list(range(NCORES))],
                    ins=[cc_in[:].opt()],
                    outs=[cc_out[:].opt()],
                )
            zT = small.tile([64, B], F32, tag="zT")
            nc.sync.dma_start(
                out=zT[:].rearrange("f (c b) -> f c b", c=NCORES),
                in_=cc_out[:].rearrange("c f b -> f c b"),
            )

            if stage == 5:
                nc.sync.dma_start(out=out[0:1, 0:B], in_=zT[0:1, :])
                return
            # ---- replicated tail: 4x (BN + relu), 3x linear, heads ----
            eps_col = small.tile([64, 1], F32, tag="eps")
            nc.vector.memset(eps_col[:], BN_EPS)
            gamma = consts_sb[0:64, C_GAMMA : C_GAMMA + 1]
            beta = consts_sb[0:64, C_BETA : C_BETA + 1]
            bva = consts_sb[0:64, C_BVA : C_BVA + 1]

            cur = zT
            node = None
            for r in range(4):
                st6 = small.tile([64, 6], F32, tag=f"st6_{r}")
                mv = small.tile([64, 2], F32, tag=f"mv_{r}")
                nc.vector.bn_stats(out=st6[:], in_=cur[:])
                nc.vector.bn_aggr(out=mv[:], in_=st6[:])
                sd = small.tile([64, 1], F32, tag=f"sd_{r}")
                nc.scalar.activation(
                    out=sd[:], in_=mv[:, 1:2], func=AF.Sqrt, bias=eps_col[:], scale=1.0
                )
                rstd = small.tile([64, 1], F32, tag=f"rstd_{r}")
                nc.vector.reciprocal(out=rstd[:], in_=sd[:])
                a = small.tile([64, 1], F32, tag=f"a_{r}")
                nc.vector.tensor_mul(a[:], rstd[:], gamma)
                mc = small.tile([64, 1], F32, tag=f"mc_{r}")
                nc.vector.tensor_mul(mc[:], mv[:, 0:1], a[:])
                cb = small.tile([64, 1], F32, tag=f"cb_{r}")
                nc.vector.tensor_sub(cb[:], beta, mc[:])
                node = small.tile([64, B], F32, tag=f"node_{r}")
                nc.scalar.activation(
                    out=node[:], in_=cur[:], func=AF.Relu, bias=cb[:], scale=a[:]
                )
                if r < 3:
                    zp = psum_small.tile([64, B], F32, tag="sps")
                    nc.tensor.matmul(zp[:], va0, node[:], start=True, stop=True)
                    nxt = small.tile([64, B], F32, tag=f"z_{r + 1}")
                    nc.scalar.activation(
                        out=nxt[:], in_=zp[:], func=AF.Identity, bias=bva, scale=1.0
                    )
                    cur = nxt

            # ---- heads (everything on partition 0: mu cols 0-31, sig 32-63) ----
            hp_mu = psum_small.tile([1, B], F32, tag="sps")
            nc.tensor.matmul(
                hp_mu[:],
                consts_sb[0:64, C_HEADS : C_HEADS + 1],
                node[:],
                start=True,
                stop=True,
            )
            hp_sig = psum_small.tile([1, B], F32, tag="sps")
            nc.tensor.matmul(
                hp_sig[:],
                consts_sb[0:64, C_HEADS + 1 : C_HEADS + 2],
                node[:],
                start=True,
                stop=True,
            )
            out_sb = small.tile([1, 2 * B], F32, tag="out_sb")
            nc.scalar.activation(
                out=out_sb[0:1, 0:B],
                in_=hp_mu[:],
                func=AF.Identity,
                bias=consts_sb[0:1, C_BMU : C_BMU + 1],
                scale=1.0,
            )
            sig_t = small.tile([1, B], F32, tag="sig_t")
            nc.scalar.activation(
                out=sig_t[:],
                in_=hp_sig[:],
                func=AF.Square,
                bias=consts_sb[0:1, C_BSIG : C_BSIG + 1],
                scale=1.0,
            )
            nc.vector.tensor_scalar_add(out_sb[0:1, B : 2 * B], sig_t[:], 0.01)
            nc.sync.dma_start(out=out, in_=out_sb[:])

          _body()

    nc.compile()
    return nc


def make_consts(inp):
    f32 = np.float32
    W1 = np.asarray(inp["W1"], f32)
    b1 = np.asarray(inp["b1"], f32)
    W2 = np.asarray(inp["W2"], f32)
    b2 = np.asarray(inp["b2"], f32)
    Wenc = np.asarray(inp["Wenc"], f32)
    benc = np.asarray(inp["benc"], f32)
    WK = np.asarray(inp["WK"], f32)
    WV = np.asarray(inp["WV"], f32)
    bV = np.asarray(inp["bV"], f32)
    wQKk = np.asarray(inp["wQKk"], f32)
    WA = np.asarray(inp["WA"], f32)
    bA = np.asarray(inp["bA"], f32)
    gamma = np.asarray(inp["gamma"], f32)
    beta = np.asarray(inp["beta"], f32)
    Wmu = np.asarray(inp["Wmu"], f32)
    bmu = np.asarray(inp["bmu"], f32)
    Wsig = np.asarray(inp["Wsig"], f32)
    bsig = np.asarray(inp["bsig"], f32)

    wk = WK.T @ wQKk  # [H]
    Wva = WA @ WV  # [H,H]
    bva = WA @ bV + bA

    c = np.zeros((128, CW), f32)
    c[0:64, C_W2 : C_W2 + 64] = W2.T
    c[64:128, C_W2 + 64 : C_W2 + 128] = W2.T
    c[0:64, C_WKE : C_WKE + 64] = wk[:, None]
    c[64:128, C_WKE + 64 : C_WKE + 128] = wk[:, None]
    c[0:9, C_W1 : C_W1 + 64] = W1.T
    c[9:18, C_W1 + 64 : C_W1 + 128] = W1.T
    c[0:64, C_WVA : C_WVA + 64] = Wva.T
    c[64:128, C_WVA : C_WVA + 64] = Wva.T
    c[0:8, C_WENC : C_WENC + 64] = Wenc.T
    c[0:64, C_HEADS] = Wmu[0]
    c[0:64, C_HEADS + 1] = Wsig[0]
    c[0:64, C_B1] = b1
    c[64:128, C_B1] = b1
    c[0:64, C_B2] = b2
    c[64:128, C_B2] = b2
    c[0:64, C_GAMMA] = gamma
    c[0:64, C_BETA] = beta
    c[0:64, C_BVA] = bva
    c[0:64, C_BENC] = benc
    c[0, C_BMU] = bmu[0]
    c[0, C_BSIG] = bsig[0]
    return c


def make_in_maps(inputs):
    f32 = np.float32
    xx = np.asarray(inputs["input_xx"], f32)
    yy = np.asarray(inputs["input_yy"], f32)
    oxx = np.asarray(inputs["output_xx"], f32)
    consts = make_consts(inputs)
    oxxT_all = np.ascontiguousarray(oxx[:, 0, :].T)  # [8, B]
    in_maps = []
    for cid in range(NCORES):
        s = slice(cid * BL, (cid + 1) * BL)
        in_maps.append(
            {
                "xx": np.ascontiguousarray(xx[s]),
                "yy": np.ascontiguousarray(yy[s]),
                "oxxT": np.ascontiguousarray(oxxT_all[:, s]),
                "consts": consts,
            }
        )
    return in_maps


_NC_CACHE = {}


def get_nc():
    if "nc" not in _NC_CACHE:
        _NC_CACHE["nc"] = build_nc()
    return _NC_CACHE["nc"]


def kernel(**inputs):
    nc = get_nc()
    in_maps = make_in_maps(inputs)
    res = bass_utils.run_bass_kernel_spmd(
        nc, in_maps, core_ids=list(range(NCORES))
    )
    out = np.asarray(res.results[0]["out"], np.float32).reshape(2 * B)
    mu_out = out[0:B].reshape(B, 1).copy()
    sig_out = out[B : 2 * B].reshape(B, 1).copy()
    return mu_out, sig_out

